# revision 6
# baseline (speedup 1.0000x reference)
"""Trainium2 Bass kernel for nn_DeepModel_70703751626759 (deep-BSDE forward sim).

v4: block-diagonal full-array (K=128, M=128) matmuls replace the 32x32
tile-position waves -- one MM per stream type per chain-step (16/step vs
72), same array throughput, far fewer instructions and no concurrency
scheduling.  The two 512-column halves are fully independent pipelines
("chains") with separate SBUF tiles and PSUM banks, emitted c0-then-c1
each step so the scheduler slides them half a step apart.  b3 is folded
into W3 via a tanh-bias ones-row (bias=20 -> tanh=1.0); sigma*dw and
gamma*DT ride the WDSA matmul via host-packed dw rows DMA'd into SCA
rows 24:29.  A PE warmup burst at init gets the HAM clock to 2.4 GHz.

Data-parallel over 8 NeuronCores: 32768 samples -> 4096/core -> 2 chains
x 4 slots x 512 samples.  Slot g of chain c = sample block (g + 4c)*512;
all tiles keep slot g's rows at partition group g (rows 32g:32g+32).
Group content: XY=[X(16);Y(16)], H=[h(20);ones(1);...], ZU=[u(8);Zv(16);
dH(8)].  PST (state) stays fp32 in a persistent PSUM bank per chain.
"""

import sys
import numpy as np

if "/opt/trn_rl_repo" not in sys.path:
    sys.path.insert(0, "/opt/trn_rl_repo")

N = 16
M = 8
T = 50
DT = 0.01
GAMMA = 0.1
SIGMA = 0.2
TAU = 0.5
H = 10
BATCH = 32768
NCORES = 8
CB = BATCH // NCORES      # 4096 samples per core
BK = 512                  # samples per slot
NCH = 2                   # chains per core

F32 = np.float32
try:
    import ml_dtypes
    BF16 = ml_dtypes.bfloat16
except ImportError:          # pragma: no cover
    BF16 = np.float32

ONES_BIAS = F32(20.0)        # tanh(20) = 1.0 to bf16/fp32 precision

# fp32 consts image (ckf) column offsets
C_B1 = 0            # 50 cols: tanh1 bias per t (b1 + t*W1row0); rows 0:20/grp
C_B2 = 50           # tanh2 bias (b2, +20.0 at row 20)
C_BY1 = 51          # Y0-MLP tanh1 bias
C_BY2 = 52          # Y0-MLP tanh2 bias (+20.0 at row 20)
C_COLS = 64


def _ct(t):
    w = 1.0 if (t == 0 or t == T - 1) else 2.0
    return 0.5 * DT * w * TAU * TAU


def _weight_blocks(inp):
    """Per-slot 32x32 (or smaller) weight blocks, v3-proven algebra."""
    A = np.asarray(inp["A"], F32)
    Bm = np.asarray(inp["Bmat"], F32)
    C = np.asarray(inp["Cmat"], F32)
    D = np.asarray(inp["Dmat"], F32)
    ZW1 = np.asarray(inp["Z_W1"], F32)
    ZW2 = np.asarray(inp["Z_W2"], F32)
    ZW3 = np.asarray(inp["Z_W3"], F32)
    PW1 = np.asarray(inp["phi_W1"], F32)
    PW2 = np.asarray(inp["phi_W2"], F32)
    PW3 = np.asarray(inp["phi_W3"], F32)
    YW1 = np.asarray(inp["Y0_W1"], F32)
    YW2 = np.asarray(inp["Y0_W2"], F32)
    YW3 = np.asarray(inp["Y0_W3"], F32)
    I16 = np.eye(16, dtype=F32)

    def blk():
        return np.zeros((32, 32), F32)

    # MLP layer 1: X rows -> [hZ(10) | hphi(10)]
    W1 = blk()
    W1[0:16, 0:10] = ZW1[1:, :]
    W1[0:16, 10:20] = PW1[1:, :]
    # MLP layer 2: H1 rows 0:20 -> ph2 rows 0:20
    W2 = blk()
    W2[0:10, 0:10] = ZW2
    W2[10:20, 10:20] = PW2
    # MLP layer 3 + b3 ones-row: H2 rows 0:21 -> ZU rows [u(0:8);Zv(8:24);dH(24:32)]
    b3u = np.asarray(inp["phi_b3"], F32)
    b3z = np.asarray(inp["Z_b3"], F32)
    W3 = blk()
    W3[10:20, 0:8] = PW3
    W3[0:10, 8:24] = ZW3
    W3[0:10, 24:32] = ZW3 @ D
    W3[10:20, 24:32] = PW3
    W3[20, 0:8] = b3u
    W3[20, 8:24] = b3z
    W3[20, 24:32] = b3u + D.T @ b3z
    # pzu init from XY: Y rows -> dH rows (Y @ Bmat)
    WZY = blk()
    WZY[16:32, 24:32] = Bm
    # state drift from XY
    WDXY = blk()
    WDXY[0:16, 0:16] = DT * A.T
    WDXY[0:16, 16:32] = -DT * I16
    WDXY[16:32, 16:32] = -DT * A
    # diffusion from SCA = dw*X (+ sigma/gamma rows handled at bd level)
    WDSA = blk()
    WDSA[0:16, 0:16] = C.T
    # state drift from ZU
    WDZU = blk()
    WDZU[0:8, 0:16] = DT * Bm.T
    WDZU[8:24, 16:32] = -DT * C
    # diffusion from SCB = dw*ZU
    WDSB = blk()
    WDSB[0:8, 0:16] = D.T
    WDSB[8:24, 16:32] = I16
    # final error: [X;Y] -> rows 0:16 = Y - X
    WE = blk()
    WE[0:16, 0:16] = -I16
    WE[16:32, 0:16] = I16
    # init: X0 rows -> X rows
    WX0 = blk()
    WX0[0:16, 0:16] = I16
    # Y0 MLP (with bY3 via ones-row 20)
    WY1 = blk()
    WY1[0:16, 0:10] = YW1
    WY2 = blk()
    WY2[0:10, 0:10] = YW2
    WY3 = blk()
    WY3[0:10, 16:32] = YW3
    WY3[20, 16:32] = np.asarray(inp["Y0_b3"], F32)
    return dict(W1=W1, W2=W2, W3=W3, WZY=WZY, WDXY=WDXY, WDSA=WDSA,
                WDZU=WDZU, WDSB=WDSB, WE=WE, WX0=WX0, WY1=WY1, WY2=WY2,
                WY3=WY3)


BD_NAMES = ["W1", "W2", "W3", "WZY", "WDXY", "WDSA", "WDZU", "WDSB",
            "WE", "WX0", "WY1", "WY2", "WY3"]


def pack_weights_bd(inp):
    """Block-diagonal [128,128] bf16 images, one per stream type."""
    wb = _weight_blocks(inp)
    out = {}
    for name in BD_NAMES:
        img = np.zeros((128, 128), F32)
        for g in range(4):
            img[32 * g: 32 * g + 32, 32 * g: 32 * g + 32] = wb[name]
        out[name] = img
    # WDSA extra: SCA group-0 rows 24:29 = [dw_s0..dw_s3, 1] -> sigma/gammaDT
    bd = out["WDSA"]
    for p in range(4):
        bd[24 + p, 32 * p: 32 * p + 16] = SIGMA
        bd[28, 32 * p: 32 * p + 16] = GAMMA * DT
    return {k: v.astype(BF16) for k, v in out.items()}


def pack_weights_f32(inp):
    """Bias image: per-partition fp32 columns."""
    Zb1 = np.asarray(inp["Z_b1"], F32)
    Pb1 = np.asarray(inp["phi_b1"], F32)
    Zb2 = np.asarray(inp["Z_b2"], F32)
    Pb2 = np.asarray(inp["phi_b2"], F32)
    Yb1 = np.asarray(inp["Y0_b1"], F32)
    Yb2 = np.asarray(inp["Y0_b2"], F32)
    ZW1 = np.asarray(inp["Z_W1"], F32)
    PW1 = np.asarray(inp["phi_W1"], F32)
    img = np.zeros((128, C_COLS), F32)
    b2 = np.concatenate([Zb2, Pb2])
    for g in range(4):
        r = 32 * g
        for t in range(T):
            tv = F32(t * DT)
            img[r: r + 10, C_B1 + t] = Zb1 + tv * ZW1[0, :]
            img[r + 10: r + 20, C_B1 + t] = Pb1 + tv * PW1[0, :]
        img[r: r + 20, C_B2] = b2
        img[r + 20, C_B2] = ONES_BIAS
        img[r: r + 10, C_BY1] = Yb1
        img[r: r + 10, C_BY2] = Yb2
        img[r + 20, C_BY2] = ONES_BIAS
    return img


def pack_x0(X0, core):
    """Per-chain X0 tiles: [2][128, 512] bf16; slot g rows 0:16 = X0 block.T"""
    base = core * CB
    out = np.zeros((NCH, 128, BK), F32)
    for c in range(NCH):
        for g in range(4):
            j = g + 4 * c
            out[c, 32 * g: 32 * g + 16, :] = \
                X0[base + BK * j: base + BK * (j + 1), :].T
    return np.ascontiguousarray(out.astype(BF16))


def pack_dwat(dw, core):
    """dwat [128, T*1024]: cols t*1024 + 512c = step t, chain c.

    rows 32g:32g+24: dw of slot (g,c) replicated (for SCA/SCB elementwise)
    rows 24:29 (group 0 only): [dw_s0, dw_s1, dw_s2, dw_s3, 1]  (sigma/gamma)
    """
    base = core * CB
    d = np.asarray(dw, F32)[:, base: base + CB, 0]      # [T, CB]
    blk = d.reshape(T, 8, BK)                           # slot j = g + 4c
    out = np.empty((128, T, NCH, BK), F32)
    for c in range(NCH):
        for g in range(4):
            r = 32 * g
            out[r: r + 32, :, c, :] = blk[None, :, g + 4 * c, :]
        for j in range(4):
            out[24 + j, :, c, :] = blk[:, j + 4 * c, :]
        out[28, :, c, :] = 1.0
        out[29: 32, :, c, :] = 0.0
    out = out.reshape(128, T * NCH * BK)
    return np.ascontiguousarray(out.astype(BF16))


# ---------------------------------------------------------------------------
# numpy emulation (validates packing + algebra; bf16 casts where HW casts)
# ---------------------------------------------------------------------------

def _b(x):
    return x.astype(BF16).astype(F32)


def emulate_core(inp, core, t_steps=T):
    bd = {k: v.astype(F32) for k, v in pack_weights_bd(inp).items()}
    ckf = pack_weights_f32(inp)
    x0p = pack_x0(np.asarray(inp["X0"], F32), core).astype(F32)
    dwat = pack_dwat(np.asarray(inp["dw"], F32), core).astype(F32)
    dwat = dwat.reshape(128, T, NCH, BK)

    def bias(col):
        return ckf[:, col: col + 1]

    lacc = np.zeros((128, 128), F32)
    eacc = np.zeros((128, 8), F32)

    for c in range(NCH):
        # init
        PST = bd["WX0"].T @ x0p[c]
        ph1 = bd["WY1"].T @ x0p[c]
        H1 = _b(np.tanh(ph1 + bias(C_BY1)))
        ph2 = bd["WY2"].T @ H1
        H2 = _b(np.tanh(ph2 + bias(C_BY2)))
        PST = PST + bd["WY3"].T @ H2

        for t in range(t_steps):
            dwt = dwat[:, t, c, :]
            XY = _b(PST)
            SCA = _b(XY * dwt)
            SCA[24:29, :] = dwt[24:29, :]            # sigma/gamma dw rows (DMA)
            ph1 = bd["W1"].T @ XY
            pzu = bd["WZY"].T @ XY
            PSTn = PST + bd["WDXY"].T @ XY + bd["WDSA"].T @ SCA
            H1 = _b(np.tanh(ph1 + bias(C_B1 + t)))
            ph2 = bd["W2"].T @ H1
            H2 = _b(np.tanh(ph2 + bias(C_B2)))
            pzu = pzu + bd["W3"].T @ H2
            ZU = _b(pzu)
            SCB = _b(ZU * dwt)
            PSTn = PSTn + bd["WDZU"].T @ ZU + bd["WDSB"].T @ SCB
            lacc[:, 2 * t + c] += np.sum(
                (F32(_ct(t)) * ZU) * ZU, axis=1)
            PST = PSTn

        XY = _b(PST)
        pe = bd["WE"].T @ XY
        eacc[:, c] = np.sum(pe * pe, axis=1)
    return lacc, eacc


def reduce_outputs(laccs, eaccs, t_steps=T):
    dh_rows = np.zeros(128, bool)
    e_rows = np.zeros(128, bool)
    for g in range(4):
        dh_rows[32 * g + 24: 32 * g + 32] = True
        e_rows[32 * g: 32 * g + 16] = True
    lc = 0.0
    lb = 0.0
    for lacc, eacc in zip(laccs, eaccs):
        lc += float(np.sum(np.asarray(lacc, np.float64)[dh_rows,
                                                        :2 * t_steps]))
        lb += float(np.sum(np.asarray(eacc, np.float64)[e_rows, 0:NCH]))
    return np.array([lb / BATCH, lc / BATCH], F32)


# ---------------------------------------------------------------------------
# device program
# ---------------------------------------------------------------------------

_BUILT = {}


def build(t_steps=T):
    if t_steps in _BUILT:
        return _BUILT[t_steps]
    from contextlib import ExitStack
    import concourse.tile as tile
    from concourse import bacc, mybir

    f32 = mybir.dt.float32
    bf16 = mybir.dt.bfloat16
    AF = mybir.ActivationFunctionType
    OP = mybir.AluOpType

    nc = bacc.Bacc("TRN2", target_bir_lowering=False, debug=False)
    dwat_d = nc.dram_tensor("dwat", [128, T * NCH * BK], bf16,
                            kind="ExternalInput").ap()
    x0p_d = nc.dram_tensor("x0p", [NCH * 128, BK], bf16,
                           kind="ExternalInput").ap()
    ckb_d = nc.dram_tensor("ckb", [len(BD_NAMES) * 128, 128], bf16,
                           kind="ExternalInput").ap()
    ckf_d = nc.dram_tensor("ckf", [128, C_COLS], f32,
                           kind="ExternalInput").ap()
    lacc_d = nc.dram_tensor("out_lacc", [128, 128], f32,
                            kind="ExternalOutput").ap()
    eacc_d = nc.dram_tensor("out_eacc", [128, 8], f32,
                            kind="ExternalOutput").ap()

    with tile.TileContext(nc) as tc, ExitStack() as ctx:
        sb = ctx.enter_context(tc.tile_pool(name="sb", bufs=1))
        dwpool = ctx.enter_context(tc.tile_pool(name="dwp", bufs=3))
        ps = ctx.enter_context(tc.tile_pool(name="ps", bufs=1, space="PSUM"))

        W = {}
        for i, name in enumerate(BD_NAMES):
            W[name] = sb.tile([128, 128], bf16, tag=f"w_{name}", name=f"w_{name}")
            nc.sync.dma_start(out=W[name][:, :],
                              in_=ckb_d[128 * i: 128 * (i + 1), :])
        ckf = sb.tile([128, C_COLS], f32, tag="ckf")
        nc.sync.dma_start(out=ckf[:, :], in_=ckf_d[:, :])

        def bias(col):
            return ckf[:, col: col + 1]

        X0SB, XY, H1, H2, ZUsb, SCA, SCB, SCR = ([None, None]
                                                 for _ in range(8))
        PST, ph1, ph2, pzu = [None, None], [None, None], [None, None], \
            [None, None]
        for c in range(NCH):
            X0SB[c] = sb.tile([128, BK], bf16, tag=f"x0_{c}", name=f"x0_{c}")
            nc.sync.dma_start(out=X0SB[c][:, :],
                              in_=x0p_d[128 * c: 128 * (c + 1), :])
            XY[c] = sb.tile([128, BK], bf16, tag=f"xy_{c}", name=f"xy_{c}")
            H1[c] = sb.tile([128, BK], bf16, tag=f"h1_{c}", name=f"h1_{c}")
            H2[c] = sb.tile([128, BK], bf16, tag=f"h2_{c}", name=f"h2_{c}")
            ZUsb[c] = sb.tile([128, BK], bf16, tag=f"zu_{c}", name=f"zu_{c}")
            SCA[c] = sb.tile([128, BK], bf16, tag=f"sca_{c}", name=f"sca_{c}")
            SCB[c] = sb.tile([128, BK], bf16, tag=f"scb_{c}", name=f"scb_{c}")
            SCR[c] = sb.tile([128, BK], bf16, tag=f"scr_{c}", name=f"scr_{c}")
            PST[c] = ps.tile([128, BK], f32, tag=f"pst_{c}", name=f"pst_{c}")
            ph1[c] = ps.tile([128, BK], f32, tag=f"ph1_{c}", name=f"ph1_{c}")
            ph2[c] = ps.tile([128, BK], f32, tag=f"ph2_{c}", name=f"ph2_{c}")
            pzu[c] = ps.tile([128, BK], f32, tag=f"pzu_{c}", name=f"pzu_{c}")
        lacc = sb.tile([128, 128], f32, tag="lacc")
        eacc = sb.tile([128, 8], f32, tag="eacc")
        nc.vector.memset(lacc[:, :], 0.0)
        nc.vector.memset(eacc[:, :], 0.0)

        def mm(out_t, w, rhs, start, stop):
            nc.tensor.matmul(out=out_t[:, :], lhsT=W[w][:, :], rhs=rhs[:, :],
                             start=start, stop=stop, skip_group_check=True)

        # ---- PE warmup burst: ~18 back-to-back MMs (~3.8us cold) so the
        # HAM clock reaches 8/8 before the real pipeline starts ----
        for c in range(NCH):
            for k in range(9):
                mm(ph1[c], "WX0", X0SB[c], True, True)

        # ---- init: PST = [X0; Y0MLP(X0)] per chain ----
        for c in range(NCH):
            mm(PST[c], "WX0", X0SB[c], True, False)
            mm(ph1[c], "WY1", X0SB[c], True, True)
            nc.scalar.activation(out=H1[c][:, :], in_=ph1[c][:, :],
                                 func=AF.Tanh, bias=bias(C_BY1))
            mm(ph2[c], "WY2", H1[c], True, True)
            nc.scalar.activation(out=H2[c][:, :], in_=ph2[c][:, :],
                                 func=AF.Tanh, bias=bias(C_BY2))
            mm(PST[c], "WY3", H2[c], False, False)

        # ---- steps: software-pipelined, chains offset half a period ----
        dwtiles = {}

        def dw_slice(t, c):
            return dwtiles[t][:, BK * c: BK * (c + 1)]

        def emit_dma(t):
            dwt = dwpool.tile([128, NCH * BK], bf16, tag="dwa", name="dwa")
            nc.sync.dma_start(
                out=dwt[:, :],
                in_=dwat_d[:, NCH * BK * t: NCH * BK * (t + 1)])
            dwtiles[t] = dwt
            if t - 2 in dwtiles:
                del dwtiles[t - 2]

        def emit_h1(c, t):
            """state copy -> MLP front: ends with W3's input H2 ready."""
            dws = dw_slice(t, c)
            nc.scalar.activation(out=XY[c][:, :], in_=PST[c][:, :],
                                 func=AF.Copy)
            nc.gpsimd.tensor_tensor(out=SCA[c][:, :], in0=XY[c][:, :],
                                    in1=dws, op=OP.mult)
            nc.sync.dma_start(out=SCA[c][24:29, :], in_=dws[24:29, :])
            mm(ph1[c], "W1", XY[c], True, True)
            mm(pzu[c], "WZY", XY[c], True, False)
            mm(PST[c], "WDXY", XY[c], False, False)
            nc.scalar.activation(out=H1[c][:, :], in_=ph1[c][:, :],
                                 func=AF.Tanh, bias=bias(C_B1 + t))
            mm(ph2[c], "W2", H1[c], True, True)
            nc.scalar.activation(out=H2[c][:, :], in_=ph2[c][:, :],
                                 func=AF.Tanh, bias=bias(C_B2))

        def emit_h2(c, t):
            """MLP back + state update tail."""
            last = (t == t_steps - 1)
            dws = dw_slice(t, c)
            mm(pzu[c], "W3", H2[c], False, True)
            # SCB = pzu * dw straight from PSUM (parallel with CAST)
            nc.vector.scalar_tensor_tensor(
                out=SCB[c][:, :], in0=pzu[c][:, :], scalar=1.0,
                in1=dws, op0=OP.mult, op1=OP.mult)
            nc.vector.tensor_copy(out=ZUsb[c][:, :], in_=pzu[c][:, :])
            mm(PST[c], "WDSA", SCA[c], False, False)
            mm(PST[c], "WDSB", SCB[c], False, False)
            mm(PST[c], "WDZU", ZUsb[c], False, last)
            nc.vector.scalar_tensor_tensor(
                out=SCR[c][:, :], in0=ZUsb[c][:, :],
                scalar=float(_ct(t)), in1=ZUsb[c][:, :],
                op0=OP.mult, op1=OP.mult,
                accum_out=lacc[:, 2 * t + c: 2 * t + c + 1])

        emit_dma(0)
        emit_h1(0, 0)
        for t in range(t_steps):
            emit_h2(0, t)
            emit_h1(1, t)
            if t + 1 < t_steps:
                emit_dma(t + 1)
                emit_h1(0, t + 1)
            emit_h2(1, t)

        # ---- final ----
        for c in range(NCH):
            nc.scalar.activation(out=XY[c][:, :], in_=PST[c][:, :],
                                 func=AF.Copy)
            mm(ph1[c], "WE", XY[c], True, True)
            nc.scalar.activation(out=SCR[c][:, :], in_=ph1[c][:, :],
                                 func=AF.Square,
                                 accum_out=eacc[:, c: c + 1])
        nc.sync.dma_start(out=lacc_d[:, :], in_=lacc[:, :])
        nc.sync.dma_start(out=eacc_d[:, :], in_=eacc[:, :])

    nc.compile()
    _BUILT[t_steps] = nc
    return nc


def make_in_maps(inputs):
    bd = pack_weights_bd(inputs)
    ckb = np.concatenate([bd[name] for name in BD_NAMES], axis=0)
    ckf = pack_weights_f32(inputs)
    X0 = np.asarray(inputs["X0"], F32)
    dw = np.asarray(inputs["dw"], F32)
    in_maps = []
    for k in range(NCORES):
        x0p = pack_x0(X0, k)
        in_maps.append({
            "dwat": pack_dwat(dw, k),
            "x0p": np.ascontiguousarray(x0p.reshape(NCH * 128, BK)),
            "ckb": np.ascontiguousarray(ckb),
            "ckf": ckf,
        })
    return in_maps


def kernel(**inputs):
    from concourse.bass_utils import run_bass_kernel_spmd

    in_maps = make_in_maps(inputs)
    nc = build(T)
    res = run_bass_kernel_spmd(nc, in_maps, core_ids=list(range(NCORES)))
    laccs = [r["out_lacc"] for r in res.results]
    eaccs = [r["out_eacc"] for r in res.results]
    return reduce_outputs(laccs, eaccs)


if __name__ == "__main__":
    print("module ok")


# revision 7
# speedup vs baseline: 1.0867x; 1.0867x over previous
"""Trainium2 Bass kernel for nn_DeepModel_70703751626759 (deep-BSDE forward sim).

v4: block-diagonal full-array (K=128, M=128) matmuls replace the 32x32
tile-position waves -- one MM per stream type per chain-step (16/step vs
72), same array throughput, far fewer instructions and no concurrency
scheduling.  The two 512-column halves are fully independent pipelines
("chains") with separate SBUF tiles and PSUM banks, emitted c0-then-c1
each step so the scheduler slides them half a step apart.  b3 is folded
into W3 via a tanh-bias ones-row (bias=20 -> tanh=1.0); sigma*dw and
gamma*DT ride the WDSA matmul via host-packed dw rows DMA'd into SCA
rows 24:29.  A PE warmup burst at init gets the HAM clock to 2.4 GHz.

Data-parallel over 8 NeuronCores: 32768 samples -> 4096/core -> 2 chains
x 4 slots x 512 samples.  Slot g of chain c = sample block (g + 4c)*512;
all tiles keep slot g's rows at partition group g (rows 32g:32g+32).
Group content: XY=[X(16);Y(16)], H=[h(20);ones(1);...], ZU=[u(8);Zv(16);
dH(8)].  PST (state) stays fp32 in a persistent PSUM bank per chain.
"""

import sys
import numpy as np

if "/opt/trn_rl_repo" not in sys.path:
    sys.path.insert(0, "/opt/trn_rl_repo")

N = 16
M = 8
T = 50
DT = 0.01
GAMMA = 0.1
SIGMA = 0.2
TAU = 0.5
H = 10
BATCH = 32768
NCORES = 8
CB = BATCH // NCORES      # 4096 samples per core
BK = 512                  # samples per slot
NCH = 2                   # chains per core

F32 = np.float32
try:
    import ml_dtypes
    BF16 = ml_dtypes.bfloat16
except ImportError:          # pragma: no cover
    BF16 = np.float32

ONES_BIAS = F32(20.0)        # tanh(20) = 1.0 to bf16/fp32 precision

# fp32 consts image (ckf) column offsets
C_B1 = 0            # 50 cols: tanh1 bias per t (b1 + t*W1row0); rows 0:20/grp
C_B2 = 50           # tanh2 bias (b2, +20.0 at row 20)
C_BY1 = 51          # Y0-MLP tanh1 bias
C_BY2 = 52          # Y0-MLP tanh2 bias (+20.0 at row 20)
C_COLS = 64


def _ct(t):
    w = 1.0 if (t == 0 or t == T - 1) else 2.0
    return 0.5 * DT * w * TAU * TAU


def _weight_blocks(inp):
    """Per-slot 32x32 (or smaller) weight blocks, v3-proven algebra."""
    A = np.asarray(inp["A"], F32)
    Bm = np.asarray(inp["Bmat"], F32)
    C = np.asarray(inp["Cmat"], F32)
    D = np.asarray(inp["Dmat"], F32)
    ZW1 = np.asarray(inp["Z_W1"], F32)
    ZW2 = np.asarray(inp["Z_W2"], F32)
    ZW3 = np.asarray(inp["Z_W3"], F32)
    PW1 = np.asarray(inp["phi_W1"], F32)
    PW2 = np.asarray(inp["phi_W2"], F32)
    PW3 = np.asarray(inp["phi_W3"], F32)
    YW1 = np.asarray(inp["Y0_W1"], F32)
    YW2 = np.asarray(inp["Y0_W2"], F32)
    YW3 = np.asarray(inp["Y0_W3"], F32)
    I16 = np.eye(16, dtype=F32)

    def blk():
        return np.zeros((32, 32), F32)

    # MLP layer 1: X rows -> [hZ(10) | hphi(10)]
    W1 = blk()
    W1[0:16, 0:10] = ZW1[1:, :]
    W1[0:16, 10:20] = PW1[1:, :]
    # MLP layer 2: H1 rows 0:20 -> ph2 rows 0:20
    W2 = blk()
    W2[0:10, 0:10] = ZW2
    W2[10:20, 10:20] = PW2
    # MLP layer 3 + b3 ones-row: H2 rows 0:21 -> ZU rows [u(0:8);Zv(8:24);dH(24:32)]
    b3u = np.asarray(inp["phi_b3"], F32)
    b3z = np.asarray(inp["Z_b3"], F32)
    W3 = blk()
    W3[10:20, 0:8] = PW3
    W3[0:10, 8:24] = ZW3
    W3[0:10, 24:32] = ZW3 @ D
    W3[10:20, 24:32] = PW3
    W3[20, 0:8] = b3u
    W3[20, 8:24] = b3z
    W3[20, 24:32] = b3u + D.T @ b3z
    # pzu init from XY: Y rows -> dH rows (Y @ Bmat)
    WZY = blk()
    WZY[16:32, 24:32] = Bm
    # state drift from XY
    WDXY = blk()
    WDXY[0:16, 0:16] = DT * A.T
    WDXY[0:16, 16:32] = -DT * I16
    WDXY[16:32, 16:32] = -DT * A
    # diffusion from SCA = dw*X (+ sigma/gamma rows handled at bd level)
    WDSA = blk()
    WDSA[0:16, 0:16] = C.T
    # state drift from ZU
    WDZU = blk()
    WDZU[0:8, 0:16] = DT * Bm.T
    WDZU[8:24, 16:32] = -DT * C
    # diffusion from SCB = dw*ZU
    WDSB = blk()
    WDSB[0:8, 0:16] = D.T
    WDSB[8:24, 16:32] = I16
    # final error: [X;Y] -> rows 0:16 = Y - X
    WE = blk()
    WE[0:16, 0:16] = -I16
    WE[16:32, 0:16] = I16
    # init: X0 rows -> X rows
    WX0 = blk()
    WX0[0:16, 0:16] = I16
    # Y0 MLP (with bY3 via ones-row 20)
    WY1 = blk()
    WY1[0:16, 0:10] = YW1
    WY2 = blk()
    WY2[0:10, 0:10] = YW2
    WY3 = blk()
    WY3[0:10, 16:32] = YW3
    WY3[20, 16:32] = np.asarray(inp["Y0_b3"], F32)
    return dict(W1=W1, W2=W2, W3=W3, WZY=WZY, WDXY=WDXY, WDSA=WDSA,
                WDZU=WDZU, WDSB=WDSB, WE=WE, WX0=WX0, WY1=WY1, WY2=WY2,
                WY3=WY3)


BD_NAMES = ["W1", "W2", "W3", "WZY", "WDXY", "WDSA", "WDZU", "WDSB",
            "WE", "WX0", "WY1", "WY2", "WY3"]


def pack_weights_bd(inp):
    """Block-diagonal [128,128] bf16 images, one per stream type."""
    wb = _weight_blocks(inp)
    out = {}
    for name in BD_NAMES:
        img = np.zeros((128, 128), F32)
        for g in range(4):
            img[32 * g: 32 * g + 32, 32 * g: 32 * g + 32] = wb[name]
        out[name] = img
    # WDSA extra: SCA group-0 rows 24:29 = [dw_s0..dw_s3, 1] -> sigma/gammaDT
    bd = out["WDSA"]
    for p in range(4):
        bd[24 + p, 32 * p: 32 * p + 16] = SIGMA
        bd[28, 32 * p: 32 * p + 16] = GAMMA * DT
    return {k: v.astype(BF16) for k, v in out.items()}


def pack_weights_f32(inp):
    """Bias image: per-partition fp32 columns."""
    Zb1 = np.asarray(inp["Z_b1"], F32)
    Pb1 = np.asarray(inp["phi_b1"], F32)
    Zb2 = np.asarray(inp["Z_b2"], F32)
    Pb2 = np.asarray(inp["phi_b2"], F32)
    Yb1 = np.asarray(inp["Y0_b1"], F32)
    Yb2 = np.asarray(inp["Y0_b2"], F32)
    ZW1 = np.asarray(inp["Z_W1"], F32)
    PW1 = np.asarray(inp["phi_W1"], F32)
    img = np.zeros((128, C_COLS), F32)
    b2 = np.concatenate([Zb2, Pb2])
    for g in range(4):
        r = 32 * g
        for t in range(T):
            tv = F32(t * DT)
            img[r: r + 10, C_B1 + t] = Zb1 + tv * ZW1[0, :]
            img[r + 10: r + 20, C_B1 + t] = Pb1 + tv * PW1[0, :]
        img[r: r + 20, C_B2] = b2
        img[r + 20, C_B2] = ONES_BIAS
        img[r: r + 10, C_BY1] = Yb1
        img[r: r + 10, C_BY2] = Yb2
        img[r + 20, C_BY2] = ONES_BIAS
    return img


def pack_x0(X0, core):
    """Per-chain X0 tiles: [2][128, 512] bf16; slot g rows 0:16 = X0 block.T"""
    base = core * CB
    out = np.zeros((NCH, 128, BK), F32)
    for c in range(NCH):
        for g in range(4):
            j = g + 4 * c
            out[c, 32 * g: 32 * g + 16, :] = \
                X0[base + BK * j: base + BK * (j + 1), :].T
    return np.ascontiguousarray(out.astype(BF16))


def pack_dwat(dw, core):
    """dwat [128, T*1024]: cols t*1024 + 512c = step t, chain c.

    rows 32g:32g+24: dw of slot (g,c) replicated (for SCA/SCB elementwise)
    rows 24:29 (group 0 only): [dw_s0, dw_s1, dw_s2, dw_s3, 1]  (sigma/gamma)
    """
    base = core * CB
    d = np.asarray(dw, F32)[:, base: base + CB, 0]      # [T, CB]
    blk = d.reshape(T, 8, BK)                           # slot j = g + 4c
    out = np.empty((128, T, NCH, BK), F32)
    for c in range(NCH):
        for g in range(4):
            r = 32 * g
            out[r: r + 32, :, c, :] = blk[None, :, g + 4 * c, :]
        for j in range(4):
            out[24 + j, :, c, :] = blk[:, j + 4 * c, :]
        out[28, :, c, :] = 1.0
        out[29: 32, :, c, :] = 0.0
    out = out.reshape(128, T * NCH * BK)
    return np.ascontiguousarray(out.astype(BF16))


# ---------------------------------------------------------------------------
# numpy emulation (validates packing + algebra; bf16 casts where HW casts)
# ---------------------------------------------------------------------------

def _b(x):
    return x.astype(BF16).astype(F32)


def emulate_core(inp, core, t_steps=T):
    bd = {k: v.astype(F32) for k, v in pack_weights_bd(inp).items()}
    ckf = pack_weights_f32(inp)
    x0p = pack_x0(np.asarray(inp["X0"], F32), core).astype(F32)
    dwat = pack_dwat(np.asarray(inp["dw"], F32), core).astype(F32)
    dwat = dwat.reshape(128, T, NCH, BK)

    def bias(col):
        return ckf[:, col: col + 1]

    lacc = np.zeros((128, 128), F32)
    eacc = np.zeros((128, 8), F32)

    for c in range(NCH):
        # init
        PST = bd["WX0"].T @ x0p[c]
        ph1 = bd["WY1"].T @ x0p[c]
        H1 = _b(np.tanh(ph1 + bias(C_BY1)))
        ph2 = bd["WY2"].T @ H1
        H2 = _b(np.tanh(ph2 + bias(C_BY2)))
        PST = PST + bd["WY3"].T @ H2

        for t in range(t_steps):
            dwt = dwat[:, t, c, :]
            XY = _b(PST)
            SCA = _b(XY * dwt)
            SCA[24:29, :] = dwt[24:29, :]            # sigma/gamma dw rows (DMA)
            ph1 = bd["W1"].T @ XY
            pzu = bd["WZY"].T @ XY
            PSTn = PST + bd["WDXY"].T @ XY + bd["WDSA"].T @ SCA
            H1 = _b(np.tanh(ph1 + bias(C_B1 + t)))
            ph2 = bd["W2"].T @ H1
            H2 = _b(np.tanh(ph2 + bias(C_B2)))
            pzu = pzu + bd["W3"].T @ H2
            ZU = _b(pzu)
            SCB = _b(ZU * dwt)
            PSTn = PSTn + bd["WDZU"].T @ ZU + bd["WDSB"].T @ SCB
            lacc[:, 2 * t + c] += np.sum(
                (F32(_ct(t)) * ZU) * ZU, axis=1)
            PST = PSTn

        XY = _b(PST)
        pe = bd["WE"].T @ XY
        eacc[:, c] = np.sum(pe * pe, axis=1)
    return lacc, eacc


def reduce_outputs(laccs, eaccs, t_steps=T):
    dh_rows = np.zeros(128, bool)
    e_rows = np.zeros(128, bool)
    for g in range(4):
        dh_rows[32 * g + 24: 32 * g + 32] = True
        e_rows[32 * g: 32 * g + 16] = True
    lc = 0.0
    lb = 0.0
    for lacc, eacc in zip(laccs, eaccs):
        lc += float(np.sum(np.asarray(lacc, np.float64)[dh_rows,
                                                        :2 * t_steps]))
        lb += float(np.sum(np.asarray(eacc, np.float64)[e_rows, 0:NCH]))
    return np.array([lb / BATCH, lc / BATCH], F32)


# ---------------------------------------------------------------------------
# device program
# ---------------------------------------------------------------------------

_BUILT = {}


def build(t_steps=T):
    if t_steps in _BUILT:
        return _BUILT[t_steps]
    from contextlib import ExitStack
    import concourse.tile as tile
    from concourse import bacc, mybir

    f32 = mybir.dt.float32
    bf16 = mybir.dt.bfloat16
    AF = mybir.ActivationFunctionType
    OP = mybir.AluOpType

    nc = bacc.Bacc("TRN2", target_bir_lowering=False, debug=False)
    dwat_d = nc.dram_tensor("dwat", [128, T * NCH * BK], bf16,
                            kind="ExternalInput").ap()
    x0p_d = nc.dram_tensor("x0p", [NCH * 128, BK], bf16,
                           kind="ExternalInput").ap()
    ckb_d = nc.dram_tensor("ckb", [len(BD_NAMES) * 128, 128], bf16,
                           kind="ExternalInput").ap()
    ckf_d = nc.dram_tensor("ckf", [128, C_COLS], f32,
                           kind="ExternalInput").ap()
    lacc_d = nc.dram_tensor("out_lacc", [128, 128], f32,
                            kind="ExternalOutput").ap()
    eacc_d = nc.dram_tensor("out_eacc", [128, 8], f32,
                            kind="ExternalOutput").ap()

    with tile.TileContext(nc) as tc, ExitStack() as ctx:
        sb = ctx.enter_context(tc.tile_pool(name="sb", bufs=1))
        dwpool = ctx.enter_context(tc.tile_pool(name="dwp", bufs=3))
        ps = ctx.enter_context(tc.tile_pool(name="ps", bufs=1, space="PSUM"))

        W = {}
        for i, name in enumerate(BD_NAMES):
            W[name] = sb.tile([128, 128], bf16, tag=f"w_{name}", name=f"w_{name}")
            nc.sync.dma_start(out=W[name][:, :],
                              in_=ckb_d[128 * i: 128 * (i + 1), :])
        ckf = sb.tile([128, C_COLS], f32, tag="ckf")
        nc.sync.dma_start(out=ckf[:, :], in_=ckf_d[:, :])

        def bias(col):
            return ckf[:, col: col + 1]

        X0SB, XY, H1, H2, ZUsb, SCA, SCB, SCR = ([None, None]
                                                 for _ in range(8))
        PST, ph1, ph2, pzu = [None, None], [None, None], [None, None], \
            [None, None]
        for c in range(NCH):
            X0SB[c] = sb.tile([128, BK], bf16, tag=f"x0_{c}", name=f"x0_{c}")
            nc.sync.dma_start(out=X0SB[c][:, :],
                              in_=x0p_d[128 * c: 128 * (c + 1), :])
            XY[c] = sb.tile([128, BK], bf16, tag=f"xy_{c}", name=f"xy_{c}")
            H1[c] = sb.tile([128, BK], bf16, tag=f"h1_{c}", name=f"h1_{c}")
            H2[c] = sb.tile([128, BK], bf16, tag=f"h2_{c}", name=f"h2_{c}")
            ZUsb[c] = sb.tile([128, BK], bf16, tag=f"zu_{c}", name=f"zu_{c}")
            SCA[c] = sb.tile([128, BK], bf16, tag=f"sca_{c}", name=f"sca_{c}")
            SCB[c] = sb.tile([128, BK], bf16, tag=f"scb_{c}", name=f"scb_{c}")
            SCR[c] = sb.tile([128, BK], bf16, tag=f"scr_{c}", name=f"scr_{c}")
            PST[c] = ps.tile([128, BK], f32, tag=f"pst_{c}", name=f"pst_{c}")
            ph1[c] = ps.tile([128, BK], f32, tag=f"ph1_{c}", name=f"ph1_{c}")
            ph2[c] = ps.tile([128, BK], f32, tag=f"ph2_{c}", name=f"ph2_{c}")
            pzu[c] = ps.tile([128, BK], f32, tag=f"pzu_{c}", name=f"pzu_{c}")
        lacc = sb.tile([128, 128], f32, tag="lacc")
        eacc = sb.tile([128, 8], f32, tag="eacc")
        nc.vector.memset(lacc[:, :], 0.0)
        nc.vector.memset(eacc[:, :], 0.0)

        def mm(out_t, w, rhs, start, stop):
            nc.tensor.matmul(out=out_t[:, :], lhsT=W[w][:, :], rhs=rhs[:, :],
                             start=start, stop=stop, skip_group_check=True)

        # ---- PE warmup burst: ~18 back-to-back MMs (~3.8us cold) so the
        # HAM clock reaches 8/8 before the real pipeline starts ----
        for c in range(NCH):
            for k in range(9):
                mm(ph1[c], "WX0", X0SB[c], True, True)

        # ---- init: PST = [X0; Y0MLP(X0)] per chain ----
        for c in range(NCH):
            mm(PST[c], "WX0", X0SB[c], True, False)
            mm(ph1[c], "WY1", X0SB[c], True, True)
            nc.scalar.activation(out=H1[c][:, :], in_=ph1[c][:, :],
                                 func=AF.Tanh, bias=bias(C_BY1))
            mm(ph2[c], "WY2", H1[c], True, True)
            nc.scalar.activation(out=H2[c][:, :], in_=ph2[c][:, :],
                                 func=AF.Tanh, bias=bias(C_BY2))
            mm(PST[c], "WY3", H2[c], False, False)

        # ---- steps: software-pipelined, chains offset half a period ----
        dwtiles = {}

        def dw_slice(t, c):
            return dwtiles[t][:, BK * c: BK * (c + 1)]

        def emit_dma(t):
            dwt = dwpool.tile([128, NCH * BK], bf16, tag="dwa", name="dwa")
            nc.sync.dma_start(
                out=dwt[:, :],
                in_=dwat_d[:, NCH * BK * t: NCH * BK * (t + 1)])
            dwtiles[t] = dwt
            if t - 2 in dwtiles:
                del dwtiles[t - 2]

        def emit_h1(c, t):
            """state copy -> MLP front: ends with W3's input H2 ready."""
            dws = dw_slice(t, c)
            if c == 0:
                nc.scalar.activation(out=XY[c][:, :], in_=PST[c][:, :],
                                     func=AF.Copy)
            else:
                nc.vector.tensor_copy(out=XY[c][:, :], in_=PST[c][:, :])
            nc.gpsimd.tensor_tensor(out=SCA[c][:, :], in0=XY[c][:, :],
                                    in1=dws, op=OP.mult)
            nc.sync.dma_start(out=SCA[c][24:29, :], in_=dws[24:29, :])
            mm(ph1[c], "W1", XY[c], True, True)
            mm(pzu[c], "WZY", XY[c], True, False)
            mm(PST[c], "WDXY", XY[c], False, False)
            nc.scalar.activation(out=H1[c][:, :], in_=ph1[c][:, :],
                                 func=AF.Tanh, bias=bias(C_B1 + t))
            mm(ph2[c], "W2", H1[c], True, True)
            nc.scalar.activation(out=H2[c][:, :], in_=ph2[c][:, :],
                                 func=AF.Tanh, bias=bias(C_B2))

        def emit_h2(c, t):
            """MLP back + state update tail."""
            last = (t == t_steps - 1)
            dws = dw_slice(t, c)
            mm(pzu[c], "W3", H2[c], False, True)
            # SCB = pzu * dw straight from PSUM (parallel with CAST)
            nc.vector.scalar_tensor_tensor(
                out=SCB[c][:, :], in0=pzu[c][:, :], scalar=1.0,
                in1=dws, op0=OP.mult, op1=OP.mult)
            nc.vector.tensor_copy(out=ZUsb[c][:, :], in_=pzu[c][:, :])
            mm(PST[c], "WDSA", SCA[c], False, False)
            mm(PST[c], "WDSB", SCB[c], False, False)
            mm(PST[c], "WDZU", ZUsb[c], False, last)
            nc.vector.scalar_tensor_tensor(
                out=SCR[c][:, :], in0=ZUsb[c][:, :],
                scalar=float(_ct(t)), in1=ZUsb[c][:, :],
                op0=OP.mult, op1=OP.mult,
                accum_out=lacc[:, 2 * t + c: 2 * t + c + 1])

        emit_dma(0)
        emit_h1(0, 0)
        for t in range(t_steps):
            emit_h2(0, t)
            emit_h1(1, t)
            if t + 1 < t_steps:
                emit_dma(t + 1)
                emit_h1(0, t + 1)
            emit_h2(1, t)

        # ---- final ----
        for c in range(NCH):
            nc.scalar.activation(out=XY[c][:, :], in_=PST[c][:, :],
                                 func=AF.Copy)
            mm(ph1[c], "WE", XY[c], True, True)
            nc.scalar.activation(out=SCR[c][:, :], in_=ph1[c][:, :],
                                 func=AF.Square,
                                 accum_out=eacc[:, c: c + 1])
        nc.sync.dma_start(out=lacc_d[:, :], in_=lacc[:, :])
        nc.sync.dma_start(out=eacc_d[:, :], in_=eacc[:, :])

    nc.compile()
    _BUILT[t_steps] = nc
    return nc


def make_in_maps(inputs):
    bd = pack_weights_bd(inputs)
    ckb = np.concatenate([bd[name] for name in BD_NAMES], axis=0)
    ckf = pack_weights_f32(inputs)
    X0 = np.asarray(inputs["X0"], F32)
    dw = np.asarray(inputs["dw"], F32)
    in_maps = []
    for k in range(NCORES):
        x0p = pack_x0(X0, k)
        in_maps.append({
            "dwat": pack_dwat(dw, k),
            "x0p": np.ascontiguousarray(x0p.reshape(NCH * 128, BK)),
            "ckb": np.ascontiguousarray(ckb),
            "ckf": ckf,
        })
    return in_maps


def kernel(**inputs):
    from concourse.bass_utils import run_bass_kernel_spmd

    in_maps = make_in_maps(inputs)
    nc = build(T)
    res = run_bass_kernel_spmd(nc, in_maps, core_ids=list(range(NCORES)))
    laccs = [r["out_lacc"] for r in res.results]
    eaccs = [r["out_eacc"] for r in res.results]
    return reduce_outputs(laccs, eaccs)


if __name__ == "__main__":
    print("module ok")


# revision 9
# speedup vs baseline: 1.1253x; 1.0355x over previous
"""Trainium2 Bass kernel for nn_DeepModel_70703751626759 (deep-BSDE forward sim).

v4: block-diagonal full-array (K=128, M=128) matmuls replace the 32x32
tile-position waves -- one MM per stream type per chain-step (16/step vs
72), same array throughput, far fewer instructions and no concurrency
scheduling.  The two 512-column halves are fully independent pipelines
("chains") with separate SBUF tiles and PSUM banks, emitted c0-then-c1
each step so the scheduler slides them half a step apart.  b3 is folded
into W3 via a tanh-bias ones-row (bias=20 -> tanh=1.0); sigma*dw and
gamma*DT ride the WDSA matmul via host-packed dw rows DMA'd into SCA
rows 24:29.  A PE warmup burst at init gets the HAM clock to 2.4 GHz.

Data-parallel over 8 NeuronCores: 32768 samples -> 4096/core -> 2 chains
x 4 slots x 512 samples.  Slot g of chain c = sample block (g + 4c)*512;
all tiles keep slot g's rows at partition group g (rows 32g:32g+32).
Group content: XY=[X(16);Y(16)], H=[h(20);ones(1);...], ZU=[u(8);Zv(16);
dH(8)].  PST (state) stays fp32 in a persistent PSUM bank per chain.
"""

import sys
import numpy as np

if "/opt/trn_rl_repo" not in sys.path:
    sys.path.insert(0, "/opt/trn_rl_repo")

N = 16
M = 8
T = 50
DT = 0.01
GAMMA = 0.1
SIGMA = 0.2
TAU = 0.5
H = 10
BATCH = 32768
NCORES = 8
CB = BATCH // NCORES      # 4096 samples per core
BK = 512                  # samples per slot
NCH = 2                   # chains per core

F32 = np.float32
try:
    import ml_dtypes
    BF16 = ml_dtypes.bfloat16
except ImportError:          # pragma: no cover
    BF16 = np.float32

ONES_BIAS = F32(20.0)        # tanh(20) = 1.0 to bf16/fp32 precision

# fp32 consts image (ckf) column offsets
C_B1 = 0            # 50 cols: tanh1 bias per t (b1 + t*W1row0); rows 0:20/grp
C_B2 = 50           # tanh2 bias (b2, +20.0 at row 20)
C_BY1 = 51          # Y0-MLP tanh1 bias
C_BY2 = 52          # Y0-MLP tanh2 bias (+20.0 at row 20)
C_COLS = 64


def _ct(t):
    w = 1.0 if (t == 0 or t == T - 1) else 2.0
    return 0.5 * DT * w * TAU * TAU


def _weight_blocks(inp):
    """Per-slot 32x32 (or smaller) weight blocks, v3-proven algebra."""
    A = np.asarray(inp["A"], F32)
    Bm = np.asarray(inp["Bmat"], F32)
    C = np.asarray(inp["Cmat"], F32)
    D = np.asarray(inp["Dmat"], F32)
    ZW1 = np.asarray(inp["Z_W1"], F32)
    ZW2 = np.asarray(inp["Z_W2"], F32)
    ZW3 = np.asarray(inp["Z_W3"], F32)
    PW1 = np.asarray(inp["phi_W1"], F32)
    PW2 = np.asarray(inp["phi_W2"], F32)
    PW3 = np.asarray(inp["phi_W3"], F32)
    YW1 = np.asarray(inp["Y0_W1"], F32)
    YW2 = np.asarray(inp["Y0_W2"], F32)
    YW3 = np.asarray(inp["Y0_W3"], F32)
    I16 = np.eye(16, dtype=F32)

    def blk():
        return np.zeros((32, 32), F32)

    # MLP layer 1: X rows -> [hZ(10) | hphi(10)]
    W1 = blk()
    W1[0:16, 0:10] = ZW1[1:, :]
    W1[0:16, 10:20] = PW1[1:, :]
    # MLP layer 2: H1 rows 0:20 -> ph2 rows 0:20
    W2 = blk()
    W2[0:10, 0:10] = ZW2
    W2[10:20, 10:20] = PW2
    # MLP layer 3 + b3 ones-row: H2 rows 0:21 -> ZU rows [u(0:8);Zv(8:24);dH(24:32)]
    b3u = np.asarray(inp["phi_b3"], F32)
    b3z = np.asarray(inp["Z_b3"], F32)
    W3 = blk()
    W3[10:20, 0:8] = PW3
    W3[0:10, 8:24] = ZW3
    W3[0:10, 24:32] = ZW3 @ D
    W3[10:20, 24:32] = PW3
    W3[20, 0:8] = b3u
    W3[20, 8:24] = b3z
    W3[20, 24:32] = b3u + D.T @ b3z
    # pzu init from XY: Y rows -> dH rows (Y @ Bmat)
    WZY = blk()
    WZY[16:32, 24:32] = Bm
    # state drift from XY
    WDXY = blk()
    WDXY[0:16, 0:16] = DT * A.T
    WDXY[0:16, 16:32] = -DT * I16
    WDXY[16:32, 16:32] = -DT * A
    # diffusion from SCA = dw*X (+ sigma/gamma rows handled at bd level)
    WDSA = blk()
    WDSA[0:16, 0:16] = C.T
    # state drift from ZU
    WDZU = blk()
    WDZU[0:8, 0:16] = DT * Bm.T
    WDZU[8:24, 16:32] = -DT * C
    # diffusion from SCB = dw*ZU
    WDSB = blk()
    WDSB[0:8, 0:16] = D.T
    WDSB[8:24, 16:32] = I16
    # final error: [X;Y] -> rows 0:16 = Y - X
    WE = blk()
    WE[0:16, 0:16] = -I16
    WE[16:32, 0:16] = I16
    # init: X0 rows -> X rows
    WX0 = blk()
    WX0[0:16, 0:16] = I16
    # Y0 MLP (with bY3 via ones-row 20)
    WY1 = blk()
    WY1[0:16, 0:10] = YW1
    WY2 = blk()
    WY2[0:10, 0:10] = YW2
    WY3 = blk()
    WY3[0:10, 16:32] = YW3
    WY3[20, 16:32] = np.asarray(inp["Y0_b3"], F32)
    return dict(W1=W1, W2=W2, W3=W3, WZY=WZY, WDXY=WDXY, WDSA=WDSA,
                WDZU=WDZU, WDSB=WDSB, WE=WE, WX0=WX0, WY1=WY1, WY2=WY2,
                WY3=WY3)


BD_NAMES = ["W1", "W2", "W3", "WZY", "WDXY", "WDSA", "WDZU", "WDSB",
            "WE", "WX0", "WY1", "WY2", "WY3"]


def pack_weights_bd(inp):
    """Block-diagonal [128,128] bf16 images, one per stream type."""
    wb = _weight_blocks(inp)
    out = {}
    for name in BD_NAMES:
        img = np.zeros((128, 128), F32)
        for g in range(4):
            img[32 * g: 32 * g + 32, 32 * g: 32 * g + 32] = wb[name]
        out[name] = img
    # WDSA extra: SCA group-0 rows 24:29 = [dw_s0..dw_s3, 1] -> sigma/gammaDT
    bd = out["WDSA"]
    for p in range(4):
        bd[24 + p, 32 * p: 32 * p + 16] = SIGMA
        bd[28, 32 * p: 32 * p + 16] = GAMMA * DT
    return {k: v.astype(BF16) for k, v in out.items()}


def pack_weights_f32(inp):
    """Bias image: per-partition fp32 columns."""
    Zb1 = np.asarray(inp["Z_b1"], F32)
    Pb1 = np.asarray(inp["phi_b1"], F32)
    Zb2 = np.asarray(inp["Z_b2"], F32)
    Pb2 = np.asarray(inp["phi_b2"], F32)
    Yb1 = np.asarray(inp["Y0_b1"], F32)
    Yb2 = np.asarray(inp["Y0_b2"], F32)
    ZW1 = np.asarray(inp["Z_W1"], F32)
    PW1 = np.asarray(inp["phi_W1"], F32)
    img = np.zeros((128, C_COLS), F32)
    b2 = np.concatenate([Zb2, Pb2])
    for g in range(4):
        r = 32 * g
        for t in range(T):
            tv = F32(t * DT)
            img[r: r + 10, C_B1 + t] = Zb1 + tv * ZW1[0, :]
            img[r + 10: r + 20, C_B1 + t] = Pb1 + tv * PW1[0, :]
        img[r: r + 20, C_B2] = b2
        img[r + 20, C_B2] = ONES_BIAS
        img[r: r + 10, C_BY1] = Yb1
        img[r: r + 10, C_BY2] = Yb2
        img[r + 20, C_BY2] = ONES_BIAS
    return img


def pack_x0(X0, core):
    """Per-chain X0 tiles: [2][128, 512] bf16; slot g rows 0:16 = X0 block.T"""
    base = core * CB
    out = np.zeros((NCH, 128, BK), F32)
    for c in range(NCH):
        for g in range(4):
            j = g + 4 * c
            out[c, 32 * g: 32 * g + 16, :] = \
                X0[base + BK * j: base + BK * (j + 1), :].T
    return np.ascontiguousarray(out.astype(BF16))


def pack_dwat(dw, core):
    """dwat [128, T*1024]: cols t*1024 + 512c = step t, chain c.

    rows 32g:32g+24: dw of slot (g,c) replicated (for SCA/SCB elementwise)
    rows 24:29 (group 0 only): [dw_s0, dw_s1, dw_s2, dw_s3, 1]  (sigma/gamma)
    """
    base = core * CB
    d = np.asarray(dw, F32)[:, base: base + CB, 0]      # [T, CB]
    blk = d.reshape(T, 8, BK)                           # slot j = g + 4c
    out = np.empty((128, T, NCH, BK), F32)
    for c in range(NCH):
        for g in range(4):
            r = 32 * g
            out[r: r + 32, :, c, :] = blk[None, :, g + 4 * c, :]
        for j in range(4):
            out[24 + j, :, c, :] = blk[:, j + 4 * c, :]
        out[28, :, c, :] = 1.0
        out[29: 32, :, c, :] = 0.0
    out = out.reshape(128, T * NCH * BK)
    return np.ascontiguousarray(out.astype(BF16))


# ---------------------------------------------------------------------------
# numpy emulation (validates packing + algebra; bf16 casts where HW casts)
# ---------------------------------------------------------------------------

def _b(x):
    return x.astype(BF16).astype(F32)


def emulate_core(inp, core, t_steps=T):
    bd = {k: v.astype(F32) for k, v in pack_weights_bd(inp).items()}
    ckf = pack_weights_f32(inp)
    x0p = pack_x0(np.asarray(inp["X0"], F32), core).astype(F32)
    dwat = pack_dwat(np.asarray(inp["dw"], F32), core).astype(F32)
    dwat = dwat.reshape(128, T, NCH, BK)

    def bias(col):
        return ckf[:, col: col + 1]

    lacc = np.zeros((128, 128), F32)
    eacc = np.zeros((128, 8), F32)

    for c in range(NCH):
        # init
        PST = bd["WX0"].T @ x0p[c]
        ph1 = bd["WY1"].T @ x0p[c]
        H1 = _b(np.tanh(ph1 + bias(C_BY1)))
        ph2 = bd["WY2"].T @ H1
        H2 = _b(np.tanh(ph2 + bias(C_BY2)))
        PST = PST + bd["WY3"].T @ H2

        for t in range(t_steps):
            dwt = dwat[:, t, c, :]
            XY = _b(PST)
            SCA = _b(XY * dwt)
            SCA[24:29, :] = dwt[24:29, :]            # sigma/gamma dw rows (DMA)
            ph1 = bd["W1"].T @ XY
            pzu = bd["WZY"].T @ XY
            PSTn = PST + bd["WDXY"].T @ XY + bd["WDSA"].T @ SCA
            H1 = _b(np.tanh(ph1 + bias(C_B1 + t)))
            ph2 = bd["W2"].T @ H1
            H2 = _b(np.tanh(ph2 + bias(C_B2)))
            pzu = pzu + bd["W3"].T @ H2
            ZU = _b(pzu)
            SCB = _b(ZU * dwt)
            PSTn = PSTn + bd["WDZU"].T @ ZU + bd["WDSB"].T @ SCB
            lacc[:, 2 * t + c] += np.sum(
                (F32(_ct(t)) * ZU) * ZU, axis=1)
            PST = PSTn

        XY = _b(PST)
        pe = bd["WE"].T @ XY
        eacc[:, c] = np.sum(pe * pe, axis=1)
    return lacc, eacc


def reduce_outputs(laccs, eaccs, t_steps=T):
    dh_rows = np.zeros(128, bool)
    e_rows = np.zeros(128, bool)
    for g in range(4):
        dh_rows[32 * g + 24: 32 * g + 32] = True
        e_rows[32 * g: 32 * g + 16] = True
    lc = 0.0
    lb = 0.0
    for lacc, eacc in zip(laccs, eaccs):
        lc += float(np.sum(np.asarray(lacc, np.float64)[dh_rows,
                                                        :2 * t_steps]))
        lb += float(np.sum(np.asarray(eacc, np.float64)[e_rows, 0:NCH]))
    return np.array([lb / BATCH, lc / BATCH], F32)


# ---------------------------------------------------------------------------
# device program
# ---------------------------------------------------------------------------

_BUILT = {}


def build(t_steps=T):
    if t_steps in _BUILT:
        return _BUILT[t_steps]
    from contextlib import ExitStack
    import concourse.tile as tile
    from concourse import bacc, mybir

    f32 = mybir.dt.float32
    bf16 = mybir.dt.bfloat16
    AF = mybir.ActivationFunctionType
    OP = mybir.AluOpType

    nc = bacc.Bacc("TRN2", target_bir_lowering=False, debug=False)
    dwat_d = nc.dram_tensor("dwat", [128, T * NCH * BK], bf16,
                            kind="ExternalInput").ap()
    x0p_d = nc.dram_tensor("x0p", [NCH * 128, BK], bf16,
                           kind="ExternalInput").ap()
    ckb_d = nc.dram_tensor("ckb", [len(BD_NAMES) * 128, 128], bf16,
                           kind="ExternalInput").ap()
    ckf_d = nc.dram_tensor("ckf", [128, C_COLS], f32,
                           kind="ExternalInput").ap()
    lacc_d = nc.dram_tensor("out_lacc", [128, 128], f32,
                            kind="ExternalOutput").ap()
    eacc_d = nc.dram_tensor("out_eacc", [128, 8], f32,
                            kind="ExternalOutput").ap()

    with tile.TileContext(nc) as tc, ExitStack() as ctx:
        sb = ctx.enter_context(tc.tile_pool(name="sb", bufs=1))
        dwpool = ctx.enter_context(tc.tile_pool(name="dwp", bufs=3))
        ps = ctx.enter_context(tc.tile_pool(name="ps", bufs=1, space="PSUM"))

        W = {}
        for i, name in enumerate(BD_NAMES):
            W[name] = sb.tile([128, 128], bf16, tag=f"w_{name}", name=f"w_{name}")
            nc.sync.dma_start(out=W[name][:, :],
                              in_=ckb_d[128 * i: 128 * (i + 1), :])
        ckf = sb.tile([128, C_COLS], f32, tag="ckf")
        nc.sync.dma_start(out=ckf[:, :], in_=ckf_d[:, :])

        def bias(col):
            return ckf[:, col: col + 1]

        X0SB, XY, H1, H2, ZUsb, SCA, SCB, SCR = ([None, None]
                                                 for _ in range(8))
        PST, ph1, ph2, pzu = [None, None], [None, None], [None, None], \
            [None, None]
        for c in range(NCH):
            X0SB[c] = sb.tile([128, BK], bf16, tag=f"x0_{c}", name=f"x0_{c}")
            nc.sync.dma_start(out=X0SB[c][:, :],
                              in_=x0p_d[128 * c: 128 * (c + 1), :])
            XY[c] = sb.tile([128, BK], bf16, tag=f"xy_{c}", name=f"xy_{c}")
            H1[c] = sb.tile([128, BK], bf16, tag=f"h1_{c}", name=f"h1_{c}")
            H2[c] = sb.tile([128, BK], bf16, tag=f"h2_{c}", name=f"h2_{c}")
            ZUsb[c] = sb.tile([128, BK], bf16, tag=f"zu_{c}", name=f"zu_{c}")
            SCA[c] = sb.tile([128, BK], bf16, tag=f"sca_{c}", name=f"sca_{c}")
            SCB[c] = sb.tile([128, BK], bf16, tag=f"scb_{c}", name=f"scb_{c}")
            SCR[c] = sb.tile([128, BK], bf16, tag=f"scr_{c}", name=f"scr_{c}")
            PST[c] = ps.tile([128, BK], f32, tag=f"pst_{c}", name=f"pst_{c}")
            ph1[c] = ps.tile([128, BK], f32, tag=f"ph1_{c}", name=f"ph1_{c}")
            ph2[c] = ps.tile([128, BK], f32, tag=f"ph2_{c}", name=f"ph2_{c}")
            pzu[c] = ps.tile([128, BK], f32, tag=f"pzu_{c}", name=f"pzu_{c}")
        lacc = sb.tile([128, 128], f32, tag="lacc")
        eacc = sb.tile([128, 8], f32, tag="eacc")
        nc.vector.memset(lacc[:, :], 0.0)
        nc.vector.memset(eacc[:, :], 0.0)

        def mm(out_t, w, rhs, start, stop):
            nc.tensor.matmul(out=out_t[:, :], lhsT=W[w][:, :], rhs=rhs[:, :],
                             start=start, stop=stop, skip_group_check=True)

        # ---- PE warmup burst: ~18 back-to-back MMs (~3.8us cold) so the
        # HAM clock reaches 8/8 before the real pipeline starts ----
        for c in range(NCH):
            for k in range(9):
                mm(ph1[c], "WX0", X0SB[c], True, True)

        # ---- init: PST = [X0; Y0MLP(X0)] per chain ----
        for c in range(NCH):
            mm(PST[c], "WX0", X0SB[c], True, False)
            mm(ph1[c], "WY1", X0SB[c], True, True)
            nc.scalar.activation(out=H1[c][:, :], in_=ph1[c][:, :],
                                 func=AF.Tanh, bias=bias(C_BY1))
            mm(ph2[c], "WY2", H1[c], True, True)
            nc.scalar.activation(out=H2[c][:, :], in_=ph2[c][:, :],
                                 func=AF.Tanh, bias=bias(C_BY2))
            mm(PST[c], "WY3", H2[c], False, False)

        # ---- steps: software-pipelined, chains offset half a period.
        # Emission order == intended execution order; engine FIFOs are
        # strict, so ops are laid out by data-readiness within each phase.
        dwtiles = {}

        def dw_slice(t, c):
            return dwtiles[t][:, BK * c: BK * (c + 1)]

        def emit_dma(t):
            dwt = dwpool.tile([128, NCH * BK], bf16, tag="dwa", name="dwa")
            nc.sync.dma_start(
                out=dwt[:, :],
                in_=dwat_d[:, NCH * BK * t: NCH * BK * (t + 1)])
            dwtiles[t] = dwt

        def emit_copy(c, t):
            if c == 0:
                nc.scalar.activation(out=XY[c][:, :], in_=PST[c][:, :],
                                     func=AF.Copy)
            else:
                nc.vector.tensor_copy(out=XY[c][:, :], in_=PST[c][:, :])

        def emit_front_a(c, t):
            """after copy: SCA + W1/WZY/WDXY + tanh1"""
            dws = dw_slice(t, c)
            nc.gpsimd.tensor_tensor(out=SCA[c][:, :], in0=XY[c][:, :],
                                    in1=dws, op=OP.mult)
            nc.sync.dma_start(out=SCA[c][24:29, :], in_=dws[24:29, :])
            mm(ph1[c], "W1", XY[c], True, True)
            mm(pzu[c], "WZY", XY[c], True, False)
            mm(PST[c], "WDXY", XY[c], False, False)
            nc.scalar.activation(out=H1[c][:, :], in_=ph1[c][:, :],
                                 func=AF.Tanh, bias=bias(C_B1 + t))

        def emit_front_b(c, t):
            mm(ph2[c], "W2", H1[c], True, True)
            nc.scalar.activation(out=H2[c][:, :], in_=ph2[c][:, :],
                                 func=AF.Tanh, bias=bias(C_B2))

        def emit_tail_pe_early(c, t):
            mm(pzu[c], "W3", H2[c], False, True)
            mm(PST[c], "WDSA", SCA[c], False, False)

        def emit_tail_dve(c, t):
            dws = dw_slice(t, c)
            nc.vector.tensor_copy(out=ZUsb[c][:, :], in_=pzu[c][:, :])
            nc.vector.tensor_tensor(out=SCB[c][:, :], in0=ZUsb[c][:, :],
                                    in1=dws, op=OP.mult)

        def emit_tail_pe_late(c, t):
            last = (t == t_steps - 1)
            mm(PST[c], "WDZU", ZUsb[c], False, False)
            mm(PST[c], "WDSB", SCB[c], False, last)

        def emit_loss(c, t):
            nc.vector.scalar_tensor_tensor(
                out=SCR[c][:, :], in0=ZUsb[c][:, :],
                scalar=float(_ct(t)), in1=ZUsb[c][:, :],
                op0=OP.mult, op1=OP.mult,
                accum_out=lacc[:, 2 * t + c: 2 * t + c + 1])

        emit_dma(0)
        emit_copy(0, 0)
        emit_front_a(0, 0)
        emit_front_b(0, 0)
        for t in range(t_steps):
            # phase A: c0 tail + c1 front, interleaved by readiness
            emit_copy(1, t)               # DVE, fires at phase start
            emit_tail_pe_early(0, t)      # W3/WDSA ready now
            emit_tail_dve(0, t)           # CAST + SCB on DVE after copy-c1
            emit_front_a(1, t)            # c1 W1-wave + tanh1
            emit_tail_pe_late(0, t)       # WDZU/WDSB once CAST/SCB land
            emit_front_b(1, t)            # c1 W2 + tanh2
            emit_loss(0, t)
            # phase B: c1 tail + c0 front (next step)
            if t + 1 < t_steps:
                emit_dma(t + 1)
                emit_copy(0, t + 1)       # ACT
            emit_tail_pe_early(1, t)
            emit_tail_dve(1, t)
            if t + 1 < t_steps:
                emit_front_a(0, t + 1)
            emit_tail_pe_late(1, t)
            if t + 1 < t_steps:
                emit_front_b(0, t + 1)
            emit_loss(1, t)

        # ---- final ----
        for c in range(NCH):
            nc.scalar.activation(out=XY[c][:, :], in_=PST[c][:, :],
                                 func=AF.Copy)
            mm(ph1[c], "WE", XY[c], True, True)
            nc.scalar.activation(out=SCR[c][:, :], in_=ph1[c][:, :],
                                 func=AF.Square,
                                 accum_out=eacc[:, c: c + 1])
        nc.sync.dma_start(out=lacc_d[:, :], in_=lacc[:, :])
        nc.sync.dma_start(out=eacc_d[:, :], in_=eacc[:, :])

    nc.compile()
    _BUILT[t_steps] = nc
    return nc


def make_in_maps(inputs):
    bd = pack_weights_bd(inputs)
    ckb = np.concatenate([bd[name] for name in BD_NAMES], axis=0)
    ckf = pack_weights_f32(inputs)
    X0 = np.asarray(inputs["X0"], F32)
    dw = np.asarray(inputs["dw"], F32)
    in_maps = []
    for k in range(NCORES):
        x0p = pack_x0(X0, k)
        in_maps.append({
            "dwat": pack_dwat(dw, k),
            "x0p": np.ascontiguousarray(x0p.reshape(NCH * 128, BK)),
            "ckb": np.ascontiguousarray(ckb),
            "ckf": ckf,
        })
    return in_maps


def kernel(**inputs):
    from concourse.bass_utils import run_bass_kernel_spmd

    in_maps = make_in_maps(inputs)
    nc = build(T)
    res = run_bass_kernel_spmd(nc, in_maps, core_ids=list(range(NCORES)))
    laccs = [r["out_lacc"] for r in res.results]
    eaccs = [r["out_eacc"] for r in res.results]
    return reduce_outputs(laccs, eaccs)


if __name__ == "__main__":
    print("module ok")
